# revision 22
# baseline (speedup 1.0000x reference)
"""BEV pooling (LSS view transform) kernel for Trainium2, 8 NeuronCores.

Problem: x (B=4, D=118, H=32, W=88, C=80) camera frustum features are pooled
into a (B, C, 360, 360) BEV grid via voxel scatter-add (segment_sum).

Structure exploited (verified at runtime from the actual inputs):
  - camera->lidar transform maps pixel (u, v, depth d): lidar (x, y) depend
    only on (u=w, d); lidar z depends only on (v=h, d).  So the BEV voxel of a
    point is a function of (d, w) alone, and the z-range keep-mask a function
    of (d, h) alone.
  - Therefore:  pooled[vox(d,w)] += sum_h zmask(d,h) * x[d,h,w,:]
  - Within a d-row, voxel ids are monotone in w, so equal-voxel groups are
    consecutive runs in w.
  - Each core's in-range voxels fit an axis-aligned rectangle of < 2^15-NI
    cells, so a per-core affine relinearization row = (vx-vx0)*sy + (vy-vy0)
    keeps every scatter index inside dma_scatter_add's int16 range (the host
    pastes the rectangle back during unshard).

Device kernel per core (core = one batch x one 44-column w-half; runs that
cross the w boundary give partial sums in each core's private grid, which
the host adds). x is fed as bf16 in (d, h, c, w) layout (halves HBM traffic
-- the streaming roofline; w innermost so run segments are contiguous).

Work is split into 4 d-phases of <=32 slabs alternating between two PSUM
partition bases, so phase p+1's matmuls never wait on phase p's copy-out:
  - stream x in [128, 3520] bf16 tiles (4 d-slabs each); PE matmul with a
    block 0/1 h-mask reduces over h into fp32 PSUM y[d, (c w)].
  - a chunked tensor_tensor_scan (state = m*state + y, fp32 state, f16 out)
    turns y into within-run prefix sums in one pass: m is 1 where slot w
    continues slot w-1's voxel run, so the full run sum lands on the run's
    LAST slot.  7 chunk segments chained via initial=prev[:, -1:] start as
    soon as each PSUM chunk is final.
  - 4 relayout copies (2 on DVE, 2 on the Activation engine) build the
    canonical dma_scatter_add source [128, 11, 80]: partition 32a+b holds
    slots (d0+b, w in [11a, 11a+11)), channels contiguous.
  - ONE dma_scatter_add per phase scatters all 1408 slots (out[idx] += in).
    Dead slots (mid-run / out-of-range / padding) go to per-token trash rows
    (distinct rows -- a shared trash row serializes the DMA's
    read-modify-write on one address).
All of this overlaps: while phase p+1 streams, phase p scans and scatters.

The grid is pre-zeroed by the runner (documented contract of
run_bass_kernel_spmd / run_bass_via_pjrt), so untouched rows read 0. It is
fp16 with rows padded to 128 ch (dma_scatter_add needs a 256B-multiple row
stride); the host upcasts, drops padding/trash, and adds w-halves.
"""

import os
import sys

import numpy as np

sys.path.insert(0, "/opt/trn_rl_repo")

# ---- problem constants (hardcoded per spec) ----
B, D, H, W, C = 4, 118, 32, 88, 80
WS = W // 2  # per-core w-column span (cores shard on batch x w-half)
CH = C  # per-core channels: full 80 (w-sharding keeps all channels)
NXX = NXY = 360
NZ = 1
V = NXX * NXY  # voxels per batch slice
DX = np.array([0.3, 0.3, 20.0], np.float32)
BX_LO = np.array([-54.0, -54.0, -10.0], np.float32)
N_CORES = 8
GROUPS = (D + 3) // 4  # 30 groups of <=4 d-slabs
# phases: (group range, d range); 32-slab aligned so hm col = d % 32
PHASES = [(0, 8, 0, 32), (8, 16, 32, 64), (16, 24, 64, 96), (24, 30, 96, D)]
NPH = len(PHASES)
NI = 32 * WS  # tokens per full scatter call (1408; phase D pads dead rows)
JW = NI // 128  # stage slots per partition (11)
# scatter calls (phase, j0, j1): the last phase is split so the first
# half's scatter DMA flies under the second half's descriptor generation
CALLS = [(0, 0, JW), (1, 0, JW), (2, 0, JW), (3, 0, 6), (3, 6, JW)]
CALL_COLS = [128 * (j1 - j0) // 16 for _, j0, j1 in CALLS]
DCOLS = 8  # idx cols for the 128-token gpsimd-library warmup call
TOTCOLS = DCOLS + sum(CALL_COLS)
NREC = 1 << 15  # device grid rows (rect + per-token trash region)
MAXROWS = NREC - NI - 2  # rect size bound for int16 indices
SENTINEL = 1 << 22  # sentinel voxel id for out-of-range slots

_NC_CACHE: dict = {}


def _host_coords(x, camera2lidar_rots, camera2lidar_trans, intrins, frustum):
    """Voxel int coords for every point, bit-identical to the reference
    (same jax ops on the cpu backend)."""
    import jax
    import jax.numpy as jnp

    cpu = jax.devices("cpu")[0]
    with jax.default_device(cpu):
        frustum = jnp.asarray(np.asarray(frustum))
        rots = jnp.asarray(np.asarray(camera2lidar_rots))
        trans = jnp.asarray(np.asarray(camera2lidar_trans))
        intr = jnp.asarray(np.asarray(intrins))
        pts = jnp.concatenate(
            [frustum[..., :2] * frustum[..., 2:3], frustum[..., 2:3]], axis=-1
        )
        combine = rots @ jnp.linalg.inv(intr)
        geom = (
            jnp.einsum("bij,dhwj->bdhwi", combine, pts)
            + trans[:, None, None, None, :]
        )
        coords = ((geom - jnp.asarray(BX_LO)) / jnp.asarray(DX)).astype(jnp.int32)
        coords = np.asarray(jax.device_get(coords))
    return coords  # (B, D, H, W, 3) int32


def _host_fallback(x, camera2lidar_rots, camera2lidar_trans, intrins, frustum):
    """Exact reference computation on host (jax cpu). Correct for arbitrary
    inputs; used only if the structure the device kernel needs doesn't hold."""
    import jax
    import jax.numpy as jnp

    cpu = jax.devices("cpu")[0]
    with jax.default_device(cpu):
        x = jnp.asarray(np.asarray(x))
        rots = jnp.asarray(np.asarray(camera2lidar_rots))
        trans = jnp.asarray(np.asarray(camera2lidar_trans))
        intr = jnp.asarray(np.asarray(intrins))
        frustum = jnp.asarray(np.asarray(frustum))
        b, d, h, w, c = x.shape
        pts = jnp.concatenate(
            [frustum[..., :2] * frustum[..., 2:3], frustum[..., 2:3]], axis=-1
        )
        combine = rots @ jnp.linalg.inv(intr)
        geom = (
            jnp.einsum("bij,dhwj->bdhwi", combine, pts)
            + trans[:, None, None, None, :]
        )
        feats = x.reshape(-1, c)
        coords = ((geom - jnp.asarray(BX_LO)) / jnp.asarray(DX)).astype(
            jnp.int32
        ).reshape(-1, 3)
        npts = feats.shape[0]
        batch_ix = jnp.repeat(jnp.arange(b, dtype=jnp.int32), npts // b)
        nx = jnp.array([NXX, NXY, NZ], jnp.int32)
        kept = jnp.all((coords >= 0) & (coords < nx), axis=-1)
        lin = ((batch_ix * NZ + coords[:, 2]) * NXX + coords[:, 0]) * NXY + coords[:, 1]
        nseg = b * NZ * NXX * NXY
        lin = jnp.where(kept, lin, nseg)
        pooled = jax.ops.segment_sum(feats, lin, num_segments=nseg + 1)[:-1]
        out = pooled.reshape(b, NZ, NXX, NXY, c).transpose(0, 1, 4, 2, 3)
        final = out.reshape(b, NZ * c, NXX, NXY)
        return np.asarray(jax.device_get(final))


def plan(coords):
    """Build per-core mask/index tables from int voxel coords.

    Returns None if the structure the device kernel relies on doesn't hold
    (caller then uses the host fallback), else a dict of planning tensors.
    """
    cx, cy, cz = coords[..., 0], coords[..., 1], coords[..., 2]
    if not (
        (cx == cx[:, :, :1, :]).all()
        and (cy == cy[:, :, :1, :]).all()
        and (cz == cz[:, :, :, :1]).all()
    ):
        return None

    vx = cx[:, :, 0, :].astype(np.int64)  # (B, D, W)
    vy = cy[:, :, 0, :].astype(np.int64)
    zk = cz[:, :, :, 0] == 0  # (B, D, H) keep mask

    inr = (vx >= 0) & (vx < NXX) & (vy >= 0) & (vy < NXY)
    slot_ids = np.arange(D * W, dtype=np.int64).reshape(1, D, W)
    vox = np.where(inr, vx * NXY + vy, SENTINEL + slot_ids)  # unique sentinels

    # Per (batch, w-half) window: runs of equal vox along the LOCAL w axis.
    # A run crossing the window boundary yields partial sums in each core's
    # private grid; the host adds the two grids, so no ownership needed.
    runcont = np.zeros((B, 2, D, WS), bool)  # slot continues previous run
    lastw = np.ones((B, 2, D, WS), bool)  # slot is its run's last
    inrw = np.zeros((B, 2, D, WS), bool)
    voxw = np.zeros((B, 2, D, WS), np.int64)
    for h in range(2):
        vw = vox[:, :, h * WS : (h + 1) * WS]
        voxw[:, h] = vw
        inrw[:, h] = inr[:, :, h * WS : (h + 1) * WS]
        runcont[:, h, :, 1:] = vw[:, :, 1:] == vw[:, :, :-1]
        lastw[:, h, :, :-1] = vw[:, :, 1:] != vw[:, :, :-1]

    scat = lastw & inrw  # run sums land on run-last slots after the scan

    # safety: within one core's window a voxel must not be scattered from
    # two different runs (the += would race across DMA engines). Fall back.
    for b in range(B):
        for h in range(2):
            v = voxw[b, h][scat[b, h]]
            if len(v) != len(np.unique(v)):
                return None

    # scan masks, tiled per channel: m[d, c*WS + w] = runcont[d, w]
    m = np.ascontiguousarray(
        np.broadcast_to(
            runcont[:, :, :, None, :].astype(np.float16), (B, 2, D, CH, WS)
        ).reshape(B, 2, D, CH * WS)
    )

    # per-core bounding rectangle of in-range voxels -> int16-safe rows
    rects = np.zeros((B, 2, 4), np.int64)  # vx0, vy0, sx, sy
    for b in range(B):
        for h in range(2):
            mk = inrw[b, h]
            if not mk.any():
                rects[b, h] = (0, 0, 0, 1)
                continue
            xs = vx[b, :, h * WS : (h + 1) * WS][mk]
            ys = vy[b, :, h * WS : (h + 1) * WS][mk]
            sx = int(xs.max() - xs.min() + 1)
            sy = int(ys.max() - ys.min() + 1)
            if sx * sy > MAXROWS:
                return None
            rects[b, h] = (int(xs.min()), int(ys.min()), sx, sy)

    # int16 idx tables for dma_scatter_add, one per CALLS entry. Token i of
    # call (p, j0, j1) reads canonical stage chunk (partition i%128, col
    # j0 + i//128); the relayout puts slot (d_local, w) at partition
    # (w//JW)*32 + d_local, col w%JW. Live slots get their rect row; dead
    # slots get a distinct trash row (sx*sy + token) -- a shared trash row
    # would serialize the DMA RMW on one address.
    tabs = []
    # warmup-call table: 128 tokens, all to trash rows (distinct)
    dtab = np.empty((B, 2, 16, DCOLS), np.int16)
    di = np.arange(128)
    for b in range(B):
        for h in range(2):
            _, _, sx, sy = rects[b, h]
            dtab[b, h, di % 16, di // 16] = (sx * sy + di).astype(np.int16)
    tabs.append(np.tile(dtab, (1, 1, 8, 1)))
    for ci, (p, j0, j1) in enumerate(CALLS):
        _, _, d0, d1 = PHASES[p]
        nic = 128 * (j1 - j0)
        ii = np.arange(nic)
        p_i, j_i = ii % 128, j0 + ii // 128
        d_loc = p_i % 32
        w_tok = JW * (p_i // 32) + j_i
        dd = d0 + d_loc
        ok = dd < d1
        ddc = np.where(ok, dd, 0)
        tab = np.empty((B, 2, nic), np.int16)
        for b in range(B):
            for h in range(2):
                vx0, vy0, sx, sy = rects[b, h]
                live = ok & scat[b, h][ddc, w_tok]
                rx = vx[b, :, h * WS : (h + 1) * WS][ddc, w_tok] - vx0
                ry = vy[b, :, h * WS : (h + 1) * WS][ddc, w_tok] - vy0
                row = rx * sy + ry
                trash = sx * sy + 128 * j0 + ii  # distinct within the phase
                tab[b, h] = np.where(live, row, trash).astype(np.int16)
        # wrap: token i lives at [i % 16, i // 16], replicated across the
        # 8 gpsimd partition groups -> (B, 2, 128, nic // 16)
        w16 = np.empty((B, 2, 16, nic // 16), np.int16)
        w16[:, :, ii % 16, ii // 16] = tab
        tabs.append(np.tile(w16, (1, 1, 8, 1)))
    idx_t = np.concatenate(tabs, axis=3)  # (B, 2, 128, TOTCOLS)

    # PE h-mask, one 32-wide block per 4-d group (phases are 32-aligned, so
    # group g's slab j accumulates into PSUM row (4g+j) % 32):
    #   hm[b, g, 32*j + h, (4*g + j) % 32] = zmask[4g+j, h]
    hm = np.zeros((B, GROUPS, 128, 32), np.float16)
    zkf = zk.astype(np.float16)
    for g in range(GROUPS):
        for j in range(min(4, D - 4 * g)):
            hm[:, g, 32 * j : 32 * j + H, (4 * g + j) % 32] = zkf[:, 4 * g + j, :]

    return {
        "hm": hm,  # (B, GROUPS, 128, 32) f16 (cast to bf16 in make_in_maps)
        "m": m,  # (B, 2, D, CH*WS) f16 scan masks
        "idx": idx_t,  # (B, 2, 128, NPH*ICOLS) i16
        "rects": rects,  # (B, 2, 4) vx0, vy0, sx, sy
    }


def build_nc():
    """Build the (single, SPMD, input-shape-static) Bass program."""
    from concourse import bacc, mybir
    from concourse import tile as tile_mod

    f32 = mybir.dt.float32
    f16 = mybir.dt.float16
    bf16 = mybir.dt.bfloat16
    i16 = mybir.dt.int16

    nc = bacc.Bacc(
        trn_type="TRN2",
        target_bir_lowering=False,
        debug=False,
        enable_asserts=False,
        num_devices=N_CORES,
        dynamic_dma_scratch_size=1 << 15,
    )
    WC = WS * CH  # 3520
    x_d = nc.dram_tensor("x_s", (D, H, CH, WS), bf16, kind="ExternalInput")
    hm_d = nc.dram_tensor("hm", (128, GROUPS * 32), bf16, kind="ExternalInput")
    m_d = nc.dram_tensor("m", (D, WC), f16, kind="ExternalInput")
    idx_d = nc.dram_tensor("idx", (128, TOTCOLS), i16, kind="ExternalInput")
    # one grid tensor per scatter call: the calls' live rows are disjoint
    # (host sums them), and separate tensors keep Tile from serializing a
    # call behind the previous call's slow RMW DMA completion
    grids = [
        nc.dram_tensor(f"grid{ci}", (NREC, 128), f16, kind="ExternalOutput")
        for ci in range(len(CALLS))
    ]

    y_t = nc.alloc_sbuf_tensor("y_t", [128, WC], f16).ap()
    y_cw = y_t.rearrange("p (c w) -> p w c", w=WS)  # strided (w, c) view

    call_off = {}
    o = DCOLS
    for ci, cols in enumerate(CALL_COLS):
        call_off[ci] = o
        o += cols

    with tile_mod.TileContext(nc) as tc:
        with (
            tc.tile_pool(name="const", bufs=1) as cp,
            tc.tile_pool(name="xp", bufs=16) as xp,
            tc.tile_pool(name="sp", bufs=4) as sp,
            tc.tile_pool(name="ps", bufs=1, space="PSUM") as pp,
        ):
            # const loads go through the Activation DGE so the Sync queue
            # starts issuing x tiles immediately
            hm_t = cp.tile([128, GROUPS * 32], bf16)
            nc.scalar.dma_start(out=hm_t[:], in_=hm_d.ap())
            m_t = cp.tile([128, WC], f16)
            nc.scalar.dma_start(out=m_t[:D, :], in_=m_d.ap())
            idx_t = cp.tile([128, TOTCOLS], i16)
            nc.scalar.dma_start(out=idx_t[:], in_=idx_d.ap())

            # each phase accumulates into its own 32-partition PSUM range:
            # no PSUM reuse, so no phase's matmuls ever wait on another
            # phase's scan-out (14080B/partition fits the 16KB banks)
            y_ps = pp.tile([128, WC], f32)
            ps_cw = y_ps.rearrange("p (c w) -> p w c", w=WS)

            def scatter(ci, s3, idx_t):
                p, j0, j1 = CALLS[ci]
                nic = 128 * (j1 - j0)
                o = call_off[ci]
                nc.gpsimd.dma_scatter_add(
                    out_ap=grids[ci].ap()[:, :CH],
                    in_ap=s3[:, j0:j1, :],
                    idxs_ap=idx_t[:, o : o + nic // 16],
                    num_idxs=nic,
                    num_idxs_reg=nic,
                    elem_size=CH,
                    elem_step=128,
                )

            # 128-token warmup scatter (targets grid0's trash rows): pulls
            # the ~16us gpsimd custom-DMA library load into the streaming
            # head instead of the first real scatter
            nc.gpsimd.dma_scatter_add(
                out_ap=grids[0].ap()[:, :CH],
                in_ap=y_t[:, :CH].rearrange("p (j e) -> p j e", e=CH),
                idxs_ap=idx_t[:, :DCOLS],
                num_idxs=128,
                num_idxs_reg=128,
                elem_size=CH,
                elem_step=128,
            )

            for p, (g0, g1, d0, d1) in enumerate(PHASES):
                # distinct PSUM ranges for the first three phases (matmul out
                # base partition must be 0/32/64); phase D reuses phase A's,
                # whose scan-out finished long before D's matmuls start
                base = (0, 32, 64, 0)[p]
                mp = d1 - d0
                for g in range(g0, g1):
                    nd = min(4, D - 4 * g)
                    rows = 32 * nd
                    xt = xp.tile([128, WC], bf16, tag="xt")
                    nc.sync.dma_start(
                        out=xt[:rows, :],
                        in_=x_d.ap()[4 * g : 4 * g + nd].rearrange(
                            "d h c w -> (d h) (c w)"
                        ),
                    )
                    for n0 in range(0, WC, 512):
                        nn = min(512, WC - n0)
                        nc.tensor.matmul(
                            out=y_ps[base : base + mp, n0 : n0 + nn],
                            lhsT=hm_t[:rows, g * 32 : g * 32 + mp],
                            rhs=xt[:rows, n0 : n0 + nn],
                            start=(g == g0),
                            stop=(g == g1 - 1),
                        )
                stage = sp.tile([128, JW * CH], f16, tag="stage")
                s3 = stage.rearrange("p (j e) -> p j e", e=CH)
                if d0 < 40:
                    # phases with w-runs: chunked segmented scan PSUM -> y_t
                    # (state = m*state + y; fp32 state, f16 out). Each
                    # 512-chunk segment starts once its PSUM cols are final;
                    # initial chains the state across chunk boundaries.
                    for n0 in range(0, WC, 512):
                        nn = min(512, WC - n0)
                        nc.vector.tensor_tensor_scan(
                            out=y_t[d0:d1, n0 : n0 + nn],
                            data0=m_t[d0:d1, n0 : n0 + nn],
                            data1=y_ps[base : base + mp, n0 : n0 + nn],
                            initial=0.0
                            if n0 == 0
                            else y_t[d0:d1, n0 - 1 : n0],
                            op0=mybir.AluOpType.mult,
                            op1=mybir.AluOpType.add,
                        )
                    # canonical stage relayout (strided (w,c) view ->
                    # contiguous chunks), split across DVE + Activation
                    for a in range(4):
                        eng = nc.vector.tensor_copy if a % 2 else nc.scalar.copy
                        eng(
                            out=s3[32 * a : 32 * a + 32, :, :],
                            in_=y_cw[d0 : d0 + 32, JW * a : JW * a + JW, :],
                        )
                    scatter(p, s3, idx_t)
                else:
                    # no runs past d=40: the scan degenerates to a copy, so
                    # relayout straight out of PSUM (cast fp32 -> f16),
                    # j-range-chunked to pipeline with the scatter calls.
                    # Rows past mp (phase D) are PSUM garbage -> trash.
                    jsplits = (
                        [(0, JW)] if p < NPH - 1 else [(0, 6), (6, JW)]
                    )
                    ci0 = p  # call index of this phase's first call
                    for k, (j0, j1) in enumerate(jsplits):
                        for a in range(4):
                            eng = (
                                nc.vector.tensor_copy
                                if a % 2
                                else nc.scalar.copy
                            )
                            eng(
                                out=s3[32 * a : 32 * a + 32, j0:j1, :],
                                in_=ps_cw[
                                    base : base + 32,
                                    JW * a + j0 : JW * a + j1,
                                    :,
                                ],
                            )
                        scatter(ci0 + k, s3, idx_t)
    nc.compile()
    return nc


def make_in_maps(x, p):
    """Per-core input dicts. Core i: batch i//2, w-half i%2."""
    import ml_dtypes

    x = np.asarray(x)
    bf16 = ml_dtypes.bfloat16
    in_maps = []
    for core in range(N_CORES):
        b, half = core // 2, core % 2
        in_maps.append(
            {
                # (D, H, C, W-slice) layout: w innermost for the run scan
                "x_s": np.ascontiguousarray(
                    x[b, :, :, half * WS : (half + 1) * WS, :].transpose(
                        0, 1, 3, 2
                    )
                ).astype(bf16),
                "hm": np.ascontiguousarray(
                    p["hm"][b].transpose(1, 0, 2).reshape(128, GROUPS * 32)
                ).astype(bf16),
                "m": p["m"][b, half],
                "idx": np.ascontiguousarray(p["idx"][b, half]),
            }
        )
    return in_maps


def assemble(results, rects):
    """results: list of 8 dicts with per-call (NREC, 128) fp16 rect grids
    (live rows disjoint across calls); sum calls, paste each core's
    rectangle, add w-halves -> (B, C, 360, 360) fp32."""
    out = np.empty((B, C, NXX, NXY), np.float32)
    for b in range(B):
        canvas = np.zeros((NXX, NXY, C), np.float32)
        for half in range(2):
            vx0, vy0, sx, sy = rects[b, half]
            res = results[2 * b + half]
            g = np.zeros((sx * sy, C), np.float32)
            for ci in range(len(CALLS)):
                g += res[f"grid{ci}"][: sx * sy, :C].astype(np.float32)
            canvas[vx0 : vx0 + sx, vy0 : vy0 + sy] += g.reshape(sx, sy, C)
        out[b] = canvas.transpose(2, 0, 1)
    return out


def _install_ntff_shim():
    """Provide antenv.axon_hooks with an NTFF profile hook driven by ctypes
    into the axon PJRT .so (the agent image's antenv lacks axon_hooks; this
    replicates trn_agent_boot's degraded-away hook). Only used when
    KERNEL_TRACE=1."""
    import contextlib
    import ctypes
    import types

    if "antenv.axon_hooks" in sys.modules:
        return
    so_path = "/opt/axon/libaxon_pjrt.so"
    if not os.path.exists(so_path):
        return
    lib = ctypes.CDLL(so_path)
    if not hasattr(lib, "axon_start_nrt_profile"):
        return
    lib.axon_start_nrt_profile.argtypes = [
        ctypes.POINTER(ctypes.c_int64),
        ctypes.c_size_t,
    ]
    lib.axon_start_nrt_profile.restype = ctypes.c_int64
    lib.axon_stop_nrt_profile.argtypes = [ctypes.c_char_p]
    lib.axon_stop_nrt_profile.restype = ctypes.c_int64

    @contextlib.contextmanager
    def _hook(output_dir, device_ids):
        import jax

        jax.devices()
        if device_ids:
            ids = (ctypes.c_int64 * len(device_ids))(*device_ids)
            rc = lib.axon_start_nrt_profile(ids, len(device_ids))
        else:
            rc = lib.axon_start_nrt_profile(None, 0)
        if rc != 0:
            raise RuntimeError(f"axon_start_nrt_profile rc={rc}")
        try:
            yield
        finally:
            n = lib.axon_stop_nrt_profile(str(output_dir).encode())
            print(f"ntff profile: {n} file(s) written to {output_dir}")

    mod = types.ModuleType("antenv.axon_hooks")
    mod.get_axon_ntff_profile_hook = lambda: _hook
    mod.set_axon_ntff_profile_hook = lambda h: None
    sys.modules["antenv.axon_hooks"] = mod


def kernel(**inputs):
    x = np.asarray(inputs["x"])
    coords = _host_coords(**inputs)
    p = plan(coords)
    if p is None:
        return _host_fallback(**inputs)

    if "v3" not in _NC_CACHE:
        _NC_CACHE["v3"] = build_nc()
    nc = _NC_CACHE["v3"]

    from concourse.bass_utils import run_bass_kernel_spmd

    trace = bool(int(os.environ.get("KERNEL_TRACE", "0")))
    trace_cores = None
    if trace:
        tc_env = os.environ.get("KERNEL_TRACE_CORES", "0")
        trace_cores = [int(t) for t in tc_env.split(",") if t != ""]
        _install_ntff_shim()
    res = run_bass_kernel_spmd(
        nc,
        make_in_maps(x, p),
        core_ids=list(range(N_CORES)),
        trace=trace,
        trace_cores=trace_cores,
    )
    kernel.last_results = res
    if res.exec_time_ns is not None:
        print(f"HW exec time: {res.exec_time_ns} ns")
    return assemble([res.results[i] for i in range(N_CORES)], p["rects"])


kernel.last_results = None
